# revision 8
# baseline (speedup 1.0000x reference)
"""Minibatch discrimination kernel for 8 trn2 NeuronCores.

reference:
    M = (x @ T).reshape(B, K, D)                       # B=1024, K=50, D=5
    abs_diffs[i,k,j] = sum_d |M[i,k,d] - M[j,k,d]|
    feat[i,k] = sum_j exp(-abs_diffs[i,k,j])
    out = concat([x, feat], axis=1)                    # [1024, 562]

Sharding (symmetric-banded): G[i,j,k] = exp(-abs_diffs) is symmetric in
(i,j).  Core c owns query rows [128c, 128c+128) and computes only the
key band [128c, 128c+640) mod B (its diagonal block + the next 4
128-blocks).  Unordered block pairs at distance 1..3 are covered exactly
once; distance-4 pairs would be covered twice (by c and c+4), so cores
4..7 get their last 128 key columns poisoned (+300 on M^T -> exp -> 0).
Every core's band contributes:
  - row sums over its band  -> feat for its own 128 rows (Exp accum_out)
  - column sums over the 4 off-diagonal chunks (PE matmul with the exp
    tile as stationary operand, ones as moving) -> feat contributions
    for rows owned by cores c+1..c+4, added host-side.

Per-core inner loop (i = 128 local rows on partitions, j = 640 band keys
on the free axis):
 - PE broadcasts row c of M^T via one-hot matmul.  T's columns are
   permuted host-side so consecutive planes land in different 32-row PE
   groups (quadrant overlap).
 - Planes d=0,2: ScalarE Abs(-psum + bias), bias = M_local[:, c].
 - Planes d=1,3,4: custom DVE op  L = |psum - M_local[:,c]| + L_prev
   (ABS_DIFF_ACC) -- abs + plane-accumulation fused in one op.
 - GpSimd adds the second scalar plane into the chain.
 - ScalarE Exp(-L1) with accum_out -> row sums.
"""

import sys

sys.path.insert(0, "/opt/trn_rl_repo")

from contextlib import ExitStack

import numpy as np

import concourse.bass as bass
import concourse.bacc as bacc
import concourse.tile as tile
from concourse import mybir
from concourse.bass_utils import run_bass_kernel_spmd

B, F = 1024, 512
K, D = 50, 5
C = K * D  # 250 columns of M
CPAD = 256  # padded (permuted) column count: 2 tiles of 128 slots
NCORES = 8
ROWS = B // NCORES  # 128 query rows per core
W = 640  # key band width per core (diag block + 4 neighbours)
NCHUNK = 4  # off-diagonal 128-chunks per band
POISON = 300.0  # added to M^T on de-duplicated key columns -> exp == 0

f32 = mybir.dt.float32
f16 = mybir.dt.float16


# ---- custom DVE op: out = |in0 - s0| + in1 ------------------------------
def _ensure_absacc():
    import concourse.dve_ops as dve_ops
    from concourse.dve_spec import C0, Spec, Src0, Src1, maxx

    for op in dve_ops.OPS:
        if op.name == "ABS_DIFF_ACC":
            return op

    def _ref(in0, in1, s0, s1, imm2):
        return (np.abs(in0.astype(np.float32) - s0) + in1).astype(np.float32)

    op = dve_ops.DveOp(
        "ABS_DIFF_ACC",
        Spec(body=maxx(Src0 - C0, C0 - Src0) + Src1, reference=_ref),
        subdim=False,
        uops_sha={"v3": "25e7d27e1dcdc09f", "v4": "1ccaf69ab942959b"},
    )
    dve_ops.OPS.append(op)  # in place: bass_utils holds a from-import binding
    dve_ops._SUB_OPCODE_FOR_NAME[op.name] = (
        dve_ops._CUSTOM_DVE_ROW_BASE + len(dve_ops.OPS) - 1
    )
    return op


ABS_DIFF_ACC = _ensure_absacc()


def _slot_assignment():
    """Map plane c -> slot s so that group(s) = (s%128)//32 == c%4."""
    slot_of = [0] * C
    nxt = {q: 0 for q in range(4)}
    for c in range(C):
        q = c % 4
        i = nxt[q]
        nxt[q] += 1
        blk, r = divmod(i, 32)
        slot_of[c] = 128 * blk + 32 * q + r
    return slot_of


SLOT_OF = _slot_assignment()


def _build_program():
    nc = bacc.Bacc("TRN2", target_bir_lowering=False)

    # per-core rotated inputs: xT columns [0, W) are this core's key band,
    # columns [0, ROWS) are its own query rows
    xT = nc.dram_tensor("xT", [F, W], f32, kind="ExternalInput").ap()
    xTloc = nc.dram_tensor("xTloc", [F, ROWS], f32, kind="ExternalInput").ap()
    Tm = nc.dram_tensor("Tm", [F, CPAD], f32, kind="ExternalInput").ap()
    onehot = nc.dram_tensor("onehot", [128, 32 * 128], f16, kind="ExternalInput").ap()
    pois = nc.dram_tensor("pois", [128, W], f16, kind="ExternalInput").ap()
    feat = nc.dram_tensor("feat", [ROWS, K], f32, kind="ExternalOutput").ap()
    csum = nc.dram_tensor("csum", [128, NCHUNK * K], f32, kind="ExternalOutput").ap()

    with tile.TileContext(nc) as tc, ExitStack() as ctx:
        const_pool = ctx.enter_context(tc.tile_pool(name="const", bufs=1))
        build_ctx = ExitStack()
        build_psum = build_ctx.enter_context(
            tc.tile_pool(name="bpsum", bufs=1, space="PSUM")
        )
        a_pool = ctx.enter_context(tc.tile_pool(name="apool", bufs=6))
        l_pool = ctx.enter_context(tc.tile_pool(name="lpool", bufs=8))
        g_pool = ctx.enter_context(tc.tile_pool(name="gpool", bufs=4))
        scratch_pool = ctx.enter_context(tc.tile_pool(name="scratch", bufs=6))

        # ---- load inputs -------------------------------------------------
        xt_sb = []
        t_sb = []
        xtl_sb = []
        for fc in range(4):
            t = const_pool.tile([128, W], f32, tag=f"xt{fc}")
            nc.sync.dma_start(out=t[:], in_=xT[128 * fc : 128 * (fc + 1), :])
            xt_sb.append(t)
            t2 = const_pool.tile([128, CPAD], f32, tag=f"tm{fc}")
            nc.sync.dma_start(out=t2[:], in_=Tm[128 * fc : 128 * (fc + 1), :])
            t_sb.append(t2)
            t3 = const_pool.tile([128, ROWS], f32, tag=f"xtl{fc}")
            nc.sync.dma_start(out=t3[:], in_=xTloc[128 * fc : 128 * (fc + 1), :])
            xtl_sb.append(t3)
        oh_sb = const_pool.tile([128, 32 * 128], f16, tag="onehot")
        nc.sync.dma_start(out=oh_sb[:], in_=onehot[:, :])
        pois_sb = const_pool.tile([128, W], f16, tag="pois")
        nc.sync.dma_start(out=pois_sb[:], in_=pois[:, :])

        ones_sb = const_pool.tile([128, 4], f16, tag="ones")
        nc.vector.memset(ones_sb[:, :], 1.0)

        # PE may carry at most one sync wait per fused matmul (walrus
        # S3_LW limit): one dummy matmul per DMA-queue sem PE will need.
        ps_dummy = build_psum.tile([128, 640], f32, tag="bld", name="ps_dummy")
        for dt_tile in (xt_sb[0], xt_sb[1], xt_sb[2], xt_sb[3], oh_sb):
            nc.tensor.matmul(
                out=ps_dummy[:, 0:512],
                lhsT=dt_tile[0:32, 0:128],
                rhs=dt_tile[0:32, 0:512],
                start=True,
                stop=True,
                tile_position=(0, 0),
            )

        # ---- build M^T (permuted slots; [256, W] as 2 tiles) -------------
        mt_sb = [
            const_pool.tile([128, W], f16, tag="mt0", name="mt0"),
            const_pool.tile([128, W], f16, tag="mt1", name="mt1"),
        ]
        for blk in range(2):
            ps = build_psum.tile([128, 640], f32, tag="bld")
            for jh, (j0, j1) in enumerate(((0, 512), (512, W))):
                for fc in range(4):
                    nc.tensor.matmul(
                        out=ps[:, j0:j1],
                        lhsT=t_sb[fc][:, 128 * blk : 128 * (blk + 1)],
                        rhs=xt_sb[fc][:, j0:j1],
                        start=(fc == 0),
                        stop=(fc == 3),
                    )
            # fp16 M^T plus poison (+300 on de-duplicated key columns)
            nc.vector.scalar_tensor_tensor(
                out=mt_sb[blk][:, :],
                in0=ps[:, :],
                scalar=1.0,
                in1=pois_sb[:, :],
                op0=mybir.AluOpType.mult,
                op1=mybir.AluOpType.add,
            )

        # ---- build M_local [128, 256] (same slot permutation) ------------
        mloc = const_pool.tile([128, CPAD], f32, tag="mloc")
        ps = build_psum.tile([128, 640], f32, tag="bld")
        for fc in range(4):
            nc.tensor.matmul(
                out=ps[:, :CPAD],
                lhsT=xtl_sb[fc][:],
                rhs=t_sb[fc][:],
                start=(fc == 0),
                stop=(fc == 3),
            )
        nc.scalar.copy(mloc[:], ps[:, :CPAD])

        build_ctx.close()  # release build psum banks before the main loop

        cs_pool = ctx.enter_context(tc.tile_pool(name="cspsum", bufs=1, space="PSUM"))
        bc_psum = ctx.enter_context(tc.tile_pool(name="bcpsum", bufs=3, space="PSUM"))

        feat_sb = const_pool.tile([128, K], f32, tag="feat")
        cs_ps = cs_pool.tile([128, NCHUNK * K], f32, tag="cs", name="cs_ps")

        # ---- main loop over the 50 kernels -------------------------------
        ex_tiles = {}
        for k in range(K):
            a0 = a2 = l1 = l2 = l3 = None
            for d in range(D):
                c = 5 * k + d
                s = SLOT_OF[c]
                blk, r = divmod(s, 128)
                bbase = (r // 32) * 32
                c0 = r % 32
                psd = bc_psum.tile([128, W], f32, tag="bc")
                for j0, j1 in ((0, 512), (512, W)):
                    nc.tensor.matmul(
                        out=psd[:, j0:j1],
                        lhsT=oh_sb[bbase : bbase + 32, 128 * c0 : 128 * (c0 + 1)],
                        rhs=mt_sb[blk][bbase : bbase + 32, j0:j1],
                        start=True,
                        stop=True,
                        tile_position=(bbase, 0),
                    )

                if d == 0:
                    a0 = a_pool.tile([128, W], f16, tag="a0")
                    nc.scalar.activation(
                        a0[:],
                        psd[:],
                        mybir.ActivationFunctionType.Abs,
                        bias=mloc[:, s : s + 1],
                        scale=-1.0,
                    )
                elif d == 1:
                    l1 = l_pool.tile([128, W], f16, tag="l")
                    nc.vector._custom_dve(
                        ABS_DIFF_ACC,
                        out=l1[:],
                        in0=psd[:],
                        in1=a0[:],
                        s0=mloc[:, s : s + 1],
                    )
                elif d == 2:
                    a2 = a_pool.tile([128, W], f16, tag="a2")
                    nc.scalar.activation(
                        a2[:],
                        psd[:],
                        mybir.ActivationFunctionType.Abs,
                        bias=mloc[:, s : s + 1],
                        scale=-1.0,
                    )
                elif d == 3:
                    l2 = l_pool.tile([128, W], f16, tag="l")
                    nc.vector._custom_dve(
                        ABS_DIFF_ACC,
                        out=l2[:],
                        in0=psd[:],
                        in1=l1[:],
                        s0=mloc[:, s : s + 1],
                    )
                else:
                    l3 = l_pool.tile([128, W], f16, tag="l")
                    nc.vector._custom_dve(
                        ABS_DIFF_ACC,
                        out=l3[:],
                        in0=psd[:],
                        in1=l2[:],
                        s0=mloc[:, s : s + 1],
                    )

            lall = g_pool.tile([128, W], f16, tag="g")
            nc.gpsimd.tensor_tensor(
                out=lall[:], in0=l3[:], in1=a2[:], op=mybir.AluOpType.add
            )

            ex = scratch_pool.tile([128, W], f16, tag="ex")
            nc.scalar.activation(
                ex[:],
                lall[:],
                mybir.ActivationFunctionType.Exp,
                bias=0.0,
                scale=-1.0,
                accum_out=feat_sb[:, k : k + 1],
            )
            ex_tiles[k] = ex

            # column sums of the 4 off-diagonal chunks: cs[p, 50*ch+k] =
            # sum_i ex[i, 128*(ch+1)+p]  (exp tile as stationary operand).
            # Deferred 2 iterations so these PE ops (which wait on exp) sit
            # behind independent plane matmuls in the PE's in-order queue.
            for kc in ([k - 2] if k >= 2 else []) + ([K - 2, K - 1] if k == K - 1 else []):
                exc = ex_tiles.pop(kc)
                for ch in range(NCHUNK):
                    nc.tensor.matmul(
                        out=cs_ps[:, K * ch + kc : K * ch + kc + 1],
                        lhsT=exc[:, 128 * (ch + 1) : 128 * (ch + 2)],
                        rhs=ones_sb[:, 0:1],
                        start=True,
                        stop=True,
                    )

        cs_sb = const_pool.tile([128, NCHUNK * K], f32, tag="cssb")
        nc.scalar.copy(cs_sb[:], cs_ps[:])
        nc.sync.dma_start(out=feat[:, :], in_=feat_sb[:, :K])
        nc.sync.dma_start(out=csum[:, :], in_=cs_sb[:, :])

    nc.compile()
    return nc


_program_cache = {}


def _get_program():
    if "nc" not in _program_cache:
        _program_cache["nc"] = _build_program()
    return _program_cache["nc"]


def _make_onehot():
    oh = np.zeros((128, 32 * 128), dtype=np.float16)
    for p in range(128):
        oh[p, (p % 32) * 128 : (p % 32 + 1) * 128] = 1.0
    return oh


def kernel(x: np.ndarray, T: np.ndarray, _trace=False, _trace_kwargs=None):
    x = np.asarray(x, dtype=np.float32)
    T = np.asarray(T, dtype=np.float32)
    nc = _get_program()

    xT_full = np.ascontiguousarray(x.T)  # [512, 1024]
    Tm_perm = np.zeros((F, CPAD), dtype=np.float32)
    Tm_perm[:, SLOT_OF] = T
    oh = _make_onehot()
    pois0 = np.zeros((128, W), dtype=np.float16)
    pois1 = np.zeros((128, W), dtype=np.float16)
    pois1[:, 512:W] = POISON
    in_maps = []
    for i in range(NCORES):
        xrot = np.roll(xT_full, -ROWS * i, axis=1)
        in_maps.append(
            {
                "xT": np.ascontiguousarray(xrot[:, :W]),
                "xTloc": np.ascontiguousarray(xrot[:, :ROWS]),
                "Tm": Tm_perm,
                "onehot": oh,
                "pois": pois0 if i < 4 else pois1,
            }
        )

    res = run_bass_kernel_spmd(
        nc,
        in_maps,
        core_ids=list(range(NCORES)),
        trace=_trace,
        **(_trace_kwargs or {}),
    )
    # row sums for own rows
    feats = np.concatenate(
        [res.results[i]["feat"] for i in range(NCORES)], axis=0
    ).astype(np.float32)
    # column-sum contributions: core c's chunk ch covers rows of core
    # (c+1+ch) mod 8
    for c in range(NCORES):
        cs = res.results[c]["csum"].astype(np.float32)  # [128, 4*K]
        for ch in range(NCHUNK):
            tgt = (c + 1 + ch) % NCORES
            feats[ROWS * tgt : ROWS * (tgt + 1), :] += cs[:, K * ch : K * (ch + 1)]
    out = np.concatenate([x, feats], axis=1)
    if _trace:
        return out, res
    return out
